# revision 1
# baseline (speedup 1.0000x reference)
"""Data-parallel axial softmax attention for 8 trn2 NeuronCores.

Strategy (per sharding hint): pure data parallel — shard x along batch
(512 -> 8 x 64), replicate the 8 projection matrices + LayerNorm params
on every core, run the full fused axial-attention forward per core via
the PJRT path on the NeuronCores, concatenate per-core outputs.

Self-contained: all shapes hardcoded (B=512, P=129, D=1024, A=32, G=4,
H=16, dk=64).
"""
import numpy as np
import jax
import jax.numpy as jnp
from functools import partial

ROPE_BASE = 10000.0
LN_EPS = 1e-5
B, P, D = 512, 129, 1024
A, G, H = 32, 4, 16
DK = D // H
N_CORES = 8


def _rope_tables_np(seq_len, dk):
    inv_freq = 1.0 / (ROPE_BASE ** (np.arange(0, dk, 2, dtype=np.float32) / dk))
    t = np.arange(seq_len, dtype=np.float32)
    freqs = np.outer(t, inv_freq)
    emb = np.concatenate([freqs, freqs], axis=-1)
    return np.cos(emb), np.sin(emb)


_COS_A, _SIN_A = _rope_tables_np(A, DK)
_COS_G, _SIN_G = _rope_tables_np(G, DK)


def _rotate_half(x):
    x1, x2 = jnp.split(x, 2, axis=-1)
    return jnp.concatenate([-x2, x1], axis=-1)


def _axial_attn(xs, Wq, bq, Wk, bk, Wv, bv, Wo, bo, cos, sin):
    # xs: [N, S, D] -> out: [N, S, D]
    N, S, _ = xs.shape

    def proj(W, b):
        return (xs @ W.T + b).reshape(N, S, H, DK).transpose(0, 2, 1, 3)

    q, k, v = proj(Wq, bq), proj(Wk, bk), proj(Wv, bv)
    q = q * cos + _rotate_half(q) * sin
    k = k * cos + _rotate_half(k) * sin
    scores = jnp.einsum('nhsd,nhtd->nhst', q, k) / jnp.sqrt(
        jnp.asarray(DK, xs.dtype))
    attn = jax.nn.softmax(scores, axis=-1)
    o = jnp.einsum('nhst,nhtd->nhsd', attn, v)
    o = o.transpose(0, 2, 1, 3).reshape(N, S, H * DK)
    return o @ Wo.T + bo


def _layernorm(x, g, b):
    mu = jnp.mean(x, axis=-1, keepdims=True)
    var = jnp.mean(jnp.square(x - mu), axis=-1, keepdims=True)
    return (x - mu) * jax.lax.rsqrt(var + LN_EPS) * g + b


def _forward_local(x, Wq_a, bq_a, Wk_a, bk_a, Wv_a, bv_a, Wo_a, bo_a,
                   Wq_f, bq_f, Wk_f, bk_f, Wv_f, bv_f, Wo_f, bo_f,
                   g_ln, b_ln, cos_a, sin_a, cos_g, sin_g):
    # x: [b_local, P, D]
    bl = x.shape[0]
    grid = x[:, 1:, :].reshape(bl, A, G, D)
    x_ant = grid.transpose(0, 2, 1, 3).reshape(bl * G, A, D)
    out_ant = _axial_attn(x_ant, Wq_a, bq_a, Wk_a, bk_a, Wv_a, bv_a,
                          Wo_a, bo_a, cos_a, sin_a)
    out_ant = out_ant.reshape(bl, G, A, D).transpose(0, 2, 1, 3)
    grid = grid + out_ant
    x_freq = _layernorm(grid, g_ln, b_ln).reshape(bl * A, G, D)
    out_freq = _axial_attn(x_freq, Wq_f, bq_f, Wk_f, bk_f, Wv_f, bv_f,
                           Wo_f, bo_f, cos_g, sin_g)
    grid = grid + out_freq.reshape(bl, A, G, D)
    delta = grid.reshape(bl, A * G, D) - x[:, 1:, :]
    cls_out = jnp.mean(delta, axis=1, keepdims=True)
    return cls_out, delta


_pmapped = jax.pmap(_forward_local, axis_name='i')
_cache = {}


def kernel(**inputs):
    x = np.ascontiguousarray(inputs['x'], dtype=np.float32)
    devs = jax.devices()[:N_CORES]
    bl = B // N_CORES

    x_sh = x.reshape(N_CORES, bl, P, D)
    names = ['Wq_a', 'bq_a', 'Wk_a', 'bk_a', 'Wv_a', 'bv_a', 'Wo_a', 'bo_a',
             'Wq_f', 'bq_f', 'Wk_f', 'bk_f', 'Wv_f', 'bv_f', 'Wo_f', 'bo_f',
             'g_ln', 'b_ln']
    reps = [np.broadcast_to(np.asarray(inputs[n], np.float32),
                            (N_CORES,) + np.asarray(inputs[n]).shape)
            for n in names]
    tabs = [np.broadcast_to(t, (N_CORES,) + t.shape)
            for t in (_COS_A, _SIN_A, _COS_G, _SIN_G)]

    cls_out, delta = _pmapped(x_sh, *reps, *tabs)
    cls_out = np.asarray(cls_out).reshape(B, 1, D)
    delta = np.asarray(delta).reshape(B, A * G, D)
    return cls_out, delta
